# revision 8
# baseline (speedup 1.0000x reference)
"""CrossViewContrastiveLoss Trainium2 kernel.

loss = f(v1^T @ v2) where v1, v2 are [131072, 256] fp32 and f is a cheap
normalize/log epilogue on the [256, 256] joint matrix.

Strategy (data-parallel over N across 8 cores):
  - core c computes partial_c = v1[rows]^T @ v2[rows] for its 16384-row
    shard as a PE GEMM streaming 32 MiB of HBM (memory-bound).
  - the shard is viewed as [4096, 1024] so every DMA descriptor line is
    4 KiB contiguous (4x fewer descriptors than the [16384, 256] view);
    each 128-partition "flat tile" then holds 4 sub-rows per partition
    and contributes 4 rank-128 matmul groups.
  - chunk schedule: tiny first chunk (descriptor gen off the critical
    path at stream start), big middle chunks, descending tail so the
    last-arriving bytes have minimal downstream compute.
  - host sums the eight 256x256 partials in float64 and runs the epilogue
    (65536 elements -- negligible next to 256 MiB of streaming).
"""

import os

import numpy as np

import concourse.bacc as bacc
import concourse.bass as bass
import concourse.mybir as mybir
import concourse.tile as tile
from concourse import bass_utils

N_FULL = 131072
K = 256
NCORES = 8
N_LOC = N_FULL // NCORES  # 16384 rows per core
P = 128
RPP = 4  # original rows packed per partition line (4 KiB)
M = RPP * K  # 1024 elements per flat row
NF = N_LOC // RPP  # 4096 flat rows per core
NT = NF // P  # 32 flat tiles of 128 flat rows
CHUNK = int(os.environ.get("CVCL_CHUNK", "2"))  # flat tiles per DMA (max)
ALPHA = 9.0
EPS = 2.220446049250313e-16

_BUILD_CACHE = {}
LAST_RESULT = None  # BassKernelResults of the most recent run (for test.py)


def _install_axon_hooks_shim():
    """bass_utils' trace path imports antenv.axon_hooks, which this image
    lacks. Provide it, wiring the ctypes NTFF hook from trn_boot when the
    axon .so supports it. Harmless no-op when tracing is off."""
    import sys
    import types

    try:
        from antenv import axon_hooks  # noqa: F401

        return
    except ImportError:
        pass
    try:
        import antenv
    except ImportError:
        return
    mod = types.ModuleType("antenv.axon_hooks")
    mod._hook = None
    mod._resolved = False

    def set_axon_ntff_profile_hook(h):
        mod._hook = h
        mod._resolved = True

    def get_axon_ntff_profile_hook():
        # lazy: only touch the axon .so when tracing is actually requested
        if not mod._resolved:
            mod._resolved = True
            try:
                from trn_agent_boot.trn_boot import _ntff_profile_via_ctypes

                so_path = "/opt/axon/libaxon_pjrt.so"
                if os.path.exists(so_path):
                    mod._hook = _ntff_profile_via_ctypes(so_path)
            except Exception:
                mod._hook = None
        return mod._hook

    mod.set_axon_ntff_profile_hook = set_axon_ntff_profile_hook
    mod.get_axon_ntff_profile_hook = get_axon_ntff_profile_hook
    sys.modules["antenv.axon_hooks"] = mod
    antenv.axon_hooks = mod


try:
    _install_axon_hooks_shim()
except Exception:
    pass


N_CHASE = 4  # trailing flat tiles streamed as half-tiles (compute chases DMA)


def _chunk_schedule():
    # [1, C, ..., C, 1]: fast stream start, short compute tail
    mid_total = NT - N_CHASE - 2
    assert mid_total % CHUNK == 0
    return [1] + [CHUNK] * (mid_total // CHUNK) + [1]


def _build():
    key = CHUNK
    if key in _BUILD_CACHE:
        return _BUILD_CACHE[key]

    nc = bacc.Bacc(
        "TRN2", target_bir_lowering=False, debug=False, num_devices=NCORES
    )
    v1 = nc.dram_tensor("v1", [NF, M], mybir.dt.float32, kind="ExternalInput")
    v2 = nc.dram_tensor("v2", [NF, M], mybir.dt.float32, kind="ExternalInput")
    out = nc.dram_tensor(
        "partial", [2, P, K], mybir.dt.float32, kind="ExternalOutput"
    )

    # [nf, m] -> [p, t, m]: flat tile t holds flat rows t*128 .. t*128+127,
    # one 4 KiB contiguous line per partition
    v1r = v1.ap().rearrange("(t p) m -> p t m", p=P)
    v2r = v2.ap().rearrange("(t p) m -> p t m", p=P)
    out_r = out.ap().rearrange("c p n -> p c n")

    sizes = _chunk_schedule()

    with tile.TileContext(nc) as tc:
        with (
            tc.tile_pool(name="io", bufs=3) as io_pool,
            tc.tile_pool(name="cv", bufs=3) as cv_pool,
            tc.tile_pool(name="acc", bufs=1, space="PSUM") as psum_pool,
            tc.tile_pool(name="res", bufs=1) as res_pool,
        ):
            # one PSUM region per 128-row half of the [256, 256] output
            ps0 = psum_pool.tile([P, K], mybir.dt.float32)
            ps1 = psum_pool.tile([P, K], mybir.dt.float32)

            off = 0
            for ci, csz in enumerate(sizes):
                sl = slice(off, off + csz)
                # single DMA queue for both tensors: strict FIFO keeps the
                # v1/v2 streams in lockstep (two queues skew ~2.5us apart,
                # stalling every chunk's matmuls on the late one).
                raw1 = io_pool.tile([P, CHUNK, M], mybir.dt.float32, tag="r1")
                raw2 = io_pool.tile([P, CHUNK, M], mybir.dt.float32, tag="r2")
                nc.sync.dma_start(raw1[:, 0:csz, :], v1r[:, sl, :])
                nc.sync.dma_start(raw2[:, 0:csz, :], v2r[:, sl, :])

                # matmul inputs must be rounded by a compute op: cast
                # v1 on ACT / v2 on DVE.
                t1 = cv_pool.tile([P, CHUNK, M], mybir.dt.bfloat16, tag="c1")
                t2 = cv_pool.tile([P, CHUNK, M], mybir.dt.bfloat16, tag="c2")
                nc.scalar.copy(t1[:, 0:csz, :], raw1[:, 0:csz, :])
                nc.vector.tensor_copy(t2[:, 0:csz, :], raw2[:, 0:csz, :])
                first = off == 0
                for t in range(csz):
                    for s in range(RPP):
                        rhs = t2[:, t, s * K : (s + 1) * K]
                        fst = first and t == 0 and s == 0
                        nc.tensor.matmul(
                            ps0[:],
                            t1[:, t, s * K : s * K + 128],
                            rhs,
                            start=fst,
                            stop=False,
                        )
                        nc.tensor.matmul(
                            ps1[:],
                            t1[:, t, s * K + 128 : s * K + 256],
                            rhs,
                            start=fst,
                            stop=False,
                        )
                off += csz

            # trailing tiles in half-tile DMAs: cast + matmuls chase the
            # stream so almost no compute remains after the last byte
            M2 = M // 2
            for tt in range(off, NT):
                for h in range(2):
                    hr1 = io_pool.tile([P, M2], mybir.dt.float32, tag="hr1")
                    hr2 = io_pool.tile([P, M2], mybir.dt.float32, tag="hr2")
                    nc.sync.dma_start(hr1[:], v1r[:, tt, h * M2 : (h + 1) * M2])
                    nc.sync.dma_start(hr2[:], v2r[:, tt, h * M2 : (h + 1) * M2])
                    hc1 = cv_pool.tile([P, M2], mybir.dt.bfloat16, tag="hc1")
                    hc2 = cv_pool.tile([P, M2], mybir.dt.bfloat16, tag="hc2")
                    nc.scalar.copy(hc1[:], hr1[:])
                    nc.vector.tensor_copy(hc2[:], hr2[:])
                    last_h = tt == NT - 1 and h == 1
                    for s2 in range(2):
                        rhs = hc2[:, s2 * K : (s2 + 1) * K]
                        stop = last_h and s2 == 1
                        nc.tensor.matmul(
                            ps0[:],
                            hc1[:, s2 * K : s2 * K + 128],
                            rhs,
                            start=False,
                            stop=stop,
                        )
                        nc.tensor.matmul(
                            ps1[:],
                            hc1[:, s2 * K + 128 : s2 * K + 256],
                            rhs,
                            start=False,
                            stop=stop,
                        )

            # ps0: copy on DVE, DMA on sync queue; ps1: copy on ACT, DMA
            # on ACT's queue -- the two output pipes overlap each other
            res0 = res_pool.tile([P, K], mybir.dt.float32, tag="o0")
            nc.vector.tensor_copy(res0[:], ps0[:])
            nc.sync.dma_start(out_r[:, 0, :], res0[:])
            res1 = res_pool.tile([P, K], mybir.dt.float32, tag="o1")
            nc.scalar.copy(res1[:], ps1[:])
            nc.scalar.dma_start(out_r[:, 1, :], res1[:])

    nc.compile()
    _BUILD_CACHE[key] = nc
    return nc


def kernel(latent_view_1, latent_view_2):
    global LAST_RESULT
    v1 = np.ascontiguousarray(np.asarray(latent_view_1, dtype=np.float32))
    v2 = np.ascontiguousarray(np.asarray(latent_view_2, dtype=np.float32))
    assert v1.shape == (N_FULL, K) and v2.shape == (N_FULL, K)

    nc = _build()
    in_maps = [
        {
            "v1": v1[c * N_LOC : (c + 1) * N_LOC].reshape(NF, M),
            "v2": v2[c * N_LOC : (c + 1) * N_LOC].reshape(NF, M),
        }
        for c in range(NCORES)
    ]
    LAST_RESULT = bass_utils.run_bass_kernel_spmd(
        nc, in_maps, core_ids=list(range(NCORES))
    )

    # host epilogue in float64 on the tiny [256, 256] joint
    p_i_j = np.zeros((K, K), dtype=np.float64)
    for r in LAST_RESULT.results:
        p_i_j += np.asarray(r["partial"], dtype=np.float64).reshape(K, K)
    p_i_j = (p_i_j + p_i_j.T) / 2.0
    p_i_j = p_i_j / p_i_j.sum()
    p_i = p_i_j.sum(axis=1, keepdims=True)
    p_j = p_i_j.sum(axis=0, keepdims=True)
    p_i_j = np.maximum(p_i_j, EPS)
    p_i = np.maximum(p_i, EPS)
    p_j = np.maximum(p_j, EPS)
    loss = -(
        p_i_j
        * (
            np.log(p_i_j)
            - (ALPHA + 1.0) * np.log(p_j)
            - (ALPHA + 1.0) * np.log(p_i)
        )
    ).sum()
    return np.array(loss, dtype=np.float32)


# revision 11
# speedup vs baseline: 1.0242x; 1.0242x over previous
"""CrossViewContrastiveLoss Trainium2 kernel.

loss = f(v1^T @ v2) where v1, v2 are [131072, 256] fp32 and f is a cheap
normalize/log epilogue on the [256, 256] joint matrix.

Strategy (data-parallel over N across 8 cores):
  - core c computes partial_c = v1[rows]^T @ v2[rows] for its 16384-row
    shard as a PE GEMM streaming 32 MiB of HBM (memory-bound).
  - the shard is viewed as [4096, 1024] so every DMA descriptor line is
    4 KiB contiguous (4x fewer descriptors than the [16384, 256] view);
    each 128-partition "flat tile" then holds 4 sub-rows per partition
    and contributes 4 rank-128 matmul groups.
  - chunk schedule: tiny first chunk (descriptor gen off the critical
    path at stream start), big middle chunks, descending tail so the
    last-arriving bytes have minimal downstream compute.
  - host sums the eight 256x256 partials in float64 and runs the epilogue
    (65536 elements -- negligible next to 256 MiB of streaming).
"""

import os

import numpy as np

import concourse.bacc as bacc
import concourse.bass as bass
import concourse.mybir as mybir
import concourse.tile as tile
from concourse import bass_utils

N_FULL = 131072
K = 256
NCORES = 8
N_LOC = N_FULL // NCORES  # 16384 rows per core
P = 128
RPP = 4  # original rows packed per partition line (4 KiB)
M = RPP * K  # 1024 elements per flat row
NF = N_LOC // RPP  # 4096 flat rows per core
NT = NF // P  # 32 flat tiles of 128 flat rows
CHUNK = int(os.environ.get("CVCL_CHUNK", "2"))  # flat tiles per DMA (max)
ALPHA = 9.0
EPS = 2.220446049250313e-16

_BUILD_CACHE = {}
LAST_RESULT = None  # BassKernelResults of the most recent run (for test.py)


def _install_axon_hooks_shim():
    """bass_utils' trace path imports antenv.axon_hooks, which this image
    lacks. Provide it, wiring the ctypes NTFF hook from trn_boot when the
    axon .so supports it. Harmless no-op when tracing is off."""
    import sys
    import types

    try:
        from antenv import axon_hooks  # noqa: F401

        return
    except ImportError:
        pass
    try:
        import antenv
    except ImportError:
        return
    mod = types.ModuleType("antenv.axon_hooks")
    mod._hook = None
    mod._resolved = False

    def set_axon_ntff_profile_hook(h):
        mod._hook = h
        mod._resolved = True

    def get_axon_ntff_profile_hook():
        # lazy: only touch the axon .so when tracing is actually requested
        if not mod._resolved:
            mod._resolved = True
            try:
                from trn_agent_boot.trn_boot import _ntff_profile_via_ctypes

                so_path = "/opt/axon/libaxon_pjrt.so"
                if os.path.exists(so_path):
                    mod._hook = _ntff_profile_via_ctypes(so_path)
            except Exception:
                mod._hook = None
        return mod._hook

    mod.set_axon_ntff_profile_hook = set_axon_ntff_profile_hook
    mod.get_axon_ntff_profile_hook = get_axon_ntff_profile_hook
    sys.modules["antenv.axon_hooks"] = mod
    antenv.axon_hooks = mod


try:
    _install_axon_hooks_shim()
except Exception:
    pass





def _build():
    key = CHUNK
    if key in _BUILD_CACHE:
        return _BUILD_CACHE[key]

    nc = bacc.Bacc(
        "TRN2", target_bir_lowering=False, debug=False, num_devices=NCORES
    )
    v1 = nc.dram_tensor("v1", [NF, M], mybir.dt.float32, kind="ExternalInput")
    v2 = nc.dram_tensor("v2", [NF, M], mybir.dt.float32, kind="ExternalInput")
    out = nc.dram_tensor(
        "partial", [2, P, K], mybir.dt.float32, kind="ExternalOutput"
    )

    # [nf, m] -> [p, t, m]: flat tile t holds flat rows t*128 .. t*128+127,
    # one 4 KiB contiguous line per partition
    v1r = v1.ap().rearrange("(t p) m -> p t m", p=P)
    v2r = v2.ap().rearrange("(t p) m -> p t m", p=P)
    out_r = out.ap().rearrange("c p n -> p c n")

    with tile.TileContext(nc) as tc:
        with (
            tc.tile_pool(name="io", bufs=3) as io_pool,
            tc.tile_pool(name="cv", bufs=3) as cv_pool,
            tc.tile_pool(name="acc", bufs=1, space="PSUM") as psum_pool,
            tc.tile_pool(name="res", bufs=1) as res_pool,
        ):
            # one PSUM region per 128-row half of the [256, 256] output
            ps0 = psum_pool.tile([P, K], mybir.dt.float32)
            ps1 = psum_pool.tile([P, K], mybir.dt.float32)

            # one DMA pair per flat tile on a single queue: strict FIFO
            # keeps the v1/v2 streams in lockstep (two queues skew ~2.5us
            # apart, stalling every tile's matmuls on the late one).
            # Per-slot casts let the matmuls chase each tile's landing, so
            # only the final tile's compute trails the stream.
            for t in range(NT):
                raw1 = io_pool.tile([P, M], mybir.dt.float32, tag="r1")
                raw2 = io_pool.tile([P, M], mybir.dt.float32, tag="r2")
                nc.sync.dma_start(raw1[:], v1r[:, t, :])
                nc.sync.dma_start(raw2[:], v2r[:, t, :])
                for s in range(RPP):
                    sk = slice(s * K, (s + 1) * K)
                    # matmul inputs must be rounded by a compute op: cast
                    # v1 on ACT / v2 on DVE.
                    t1 = cv_pool.tile([P, K], mybir.dt.bfloat16, tag=f"c1{s}")
                    t2 = cv_pool.tile([P, K], mybir.dt.bfloat16, tag=f"c2{s}")
                    nc.scalar.copy(t1[:], raw1[:, sk])
                    nc.vector.tensor_copy(t2[:], raw2[:, sk])
                    fst = t == 0 and s == 0
                    stop = t == NT - 1 and s == RPP - 1
                    nc.tensor.matmul(
                        ps0[:], t1[:, 0:128], t2[:], start=fst, stop=stop
                    )
                    nc.tensor.matmul(
                        ps1[:], t1[:, 128:256], t2[:], start=fst, stop=stop
                    )

            # ps0: copy on DVE, DMA on sync queue; ps1: copy on ACT, DMA
            # on ACT's queue -- the two output pipes overlap each other
            res0 = res_pool.tile([P, K], mybir.dt.float32, tag="o0")
            nc.vector.tensor_copy(res0[:], ps0[:])
            nc.sync.dma_start(out_r[:, 0, :], res0[:])
            res1 = res_pool.tile([P, K], mybir.dt.float32, tag="o1")
            nc.scalar.copy(res1[:], ps1[:])
            nc.scalar.dma_start(out_r[:, 1, :], res1[:])

    nc.compile()
    _BUILD_CACHE[key] = nc
    return nc


def kernel(latent_view_1, latent_view_2):
    global LAST_RESULT
    v1 = np.ascontiguousarray(np.asarray(latent_view_1, dtype=np.float32))
    v2 = np.ascontiguousarray(np.asarray(latent_view_2, dtype=np.float32))
    assert v1.shape == (N_FULL, K) and v2.shape == (N_FULL, K)

    nc = _build()
    in_maps = [
        {
            "v1": v1[c * N_LOC : (c + 1) * N_LOC].reshape(NF, M),
            "v2": v2[c * N_LOC : (c + 1) * N_LOC].reshape(NF, M),
        }
        for c in range(NCORES)
    ]
    LAST_RESULT = bass_utils.run_bass_kernel_spmd(
        nc, in_maps, core_ids=list(range(NCORES))
    )

    # host epilogue in float64 on the tiny [256, 256] joint
    p_i_j = np.zeros((K, K), dtype=np.float64)
    for r in LAST_RESULT.results:
        p_i_j += np.asarray(r["partial"], dtype=np.float64).reshape(K, K)
    p_i_j = (p_i_j + p_i_j.T) / 2.0
    p_i_j = p_i_j / p_i_j.sum()
    p_i = p_i_j.sum(axis=1, keepdims=True)
    p_j = p_i_j.sum(axis=0, keepdims=True)
    p_i_j = np.maximum(p_i_j, EPS)
    p_i = np.maximum(p_i, EPS)
    p_j = np.maximum(p_j, EPS)
    loss = -(
        p_i_j
        * (
            np.log(p_i_j)
            - (ALPHA + 1.0) * np.log(p_j)
            - (ALPHA + 1.0) * np.log(p_i)
        )
    ).sum()
    return np.array(loss, dtype=np.float32)
